# revision 21
# baseline (speedup 1.0000x reference)
# Trainium2 Bass kernel for nn_FDM_3899830304921 (feature-map cosine-sim
# dual-softmax transport), data-parallel over the batch dim on 8 NeuronCores.
#
# Math per batch (c=512, n=m=784):
#   r1[n] = 1/||f1_:n||, r2[m] = 1/||f2_:m||
#   E[n,m]   = exp(-r1[n] * sum_c f1[c,n] * (f2[c,m]*r2[m]))   (= exp(-cos))
#   new_fm2  = (f1 @ E) * (0.001/colsum(E))      (softmax over n folded in)
#   new_fm1  = (f2 @ E^T) * (0.001/rowsum(E))    (softmax over m folded in)
#
# Engine assignment (per batch):
#   PE:   gram (f1^T f2), colsum ones-matmul, P1/P2 output matmuls,
#         14 tiny column->row transposes. All matmul operands bf16.
#   DMA:  IO + all big transposes (fT1, fT2, E^T) via the XBAR
#         dma_start_transpose path (2-byte dtype, 16x128 tiles).
#   ACT:  f32->bf16 input conversion, exp (w/ fused rowsum accumulation),
#         sqrt, tiny row copies. Only Exp+Sqrt use act tables.
#   DVE:  fused square+reduce for norms (tensor_tensor_reduce),
#         reciprocals, f2 prescale, output postscale multiplies.
#   Pool: pad memsets + partition_broadcast of the three scale rows.
#
# n/m are padded 784->896 (7x128) only where the XBAR transpose needs
# multiples of 128; pad lanes are zeroed or provably never read.
import sys

if "/opt/trn_rl_repo" not in sys.path:
    sys.path.insert(0, "/opt/trn_rl_repo")

import numpy as np

B_TOTAL = 32
B_PER_CORE = 4
N_CORES = 8
C = 512
N = 784  # 28*28, both spatial dims
NPAD = 896  # 7*128, for XBAR dma transposes
FACTOR = 0.001

# n (and m) tiling: 6 tiles of 128 + one of 16
NT = [(0, 128), (128, 128), (256, 128), (384, 128), (512, 128), (640, 128), (768, 16)]
# free-dim split of 784 into PSUM-bank-sized pieces
HALVES = [(0, 512), (512, 272)]

_BUILT = {}


def _build(nbatch, enable_asserts=False):
    key = (nbatch, enable_asserts)
    if key in _BUILT:
        return _BUILT[key]

    import concourse.bass as bass
    import concourse.tile as tile
    from concourse import bacc, mybir
    from concourse.masks import make_identity

    f32 = mybir.dt.float32
    bf16 = mybir.dt.bfloat16
    fp8 = mybir.dt.float8e4
    AF = mybir.ActivationFunctionType
    ALU = mybir.AluOpType

    nc = bacc.Bacc("TRN2", target_bir_lowering=False, debug=False,
                   enable_asserts=enable_asserts, num_devices=N_CORES)
    fm1 = nc.dram_tensor("fm1", [nbatch, C, N], f32, kind="ExternalInput").ap()
    fm2 = nc.dram_tensor("fm2", [nbatch, C, N], f32, kind="ExternalInput").ap()
    o1 = nc.dram_tensor("o1", [nbatch, C, N], f32, kind="ExternalOutput").ap()
    o2 = nc.dram_tensor("o2", [nbatch, C, N], f32, kind="ExternalOutput").ap()

    with tile.TileContext(nc) as tc:
        with (
            tc.tile_pool(name="sb", bufs=2) as sb,
            tc.tile_pool(name="ps", bufs=4, space="PSUM") as ps,
        ):
            ident = sb.tile([128, 128], f32, tag="ident", bufs=1)
            make_identity(nc, ident[:])
            onesb = sb.tile([128, 1], bf16, tag="onesb", bufs=1)
            nc.vector.memset(onesb[:], 1.0)

            def row_to_bcast(rcp_row_ps, tagstem):
                """[1,N] PSUM row holding 1/denom -> broadcast FACTOR/denom
                to [128,N] bf16 (FACTOR folded into the copy scale)."""
                rb = sb.tile([1, N], bf16, tag="rowb", bufs=4)
                nc.scalar.mul(rb[:1, :], rcp_row_ps, FACTOR)
                out = sb.tile([128, N], bf16, tag=tagstem + "B", bufs=2)
                nc.gpsimd.partition_broadcast(out[:], rb[:1, :])
                return out

            def prep(b):
                """Load batch b, convert to bf16, transpose via XBAR DMA,
                compute norms and the r2-prescaled f2. The f2 path is
                emitted first: the gram needs f2s (prescale) while the f1
                transpose/norms are only needed later (exp scale, P2)."""
                fB = sb.tile([128, 4, N], f32, tag="fB", bufs=2)
                nc.sync.dma_start(
                    out=fB[:], in_=fm2[b].rearrange("(t p) n -> p t n", p=128))
                fA = sb.tile([128, 4, N], f32, tag="fA", bufs=2)
                nc.sync.dma_start(
                    out=fA[:], in_=fm1[b].rearrange("(t p) n -> p t n", p=128))

                f1w = sb.tile([128, 4, NPAD], bf16, tag="f1w", bufs=2)
                f2w = sb.tile([128, 4, NPAD], bf16, tag="f2w", bufs=2)
                nc.gpsimd.memset(f2w[:, :, N:], 0.0)
                nc.gpsimd.memset(f1w[:, :, N:], 0.0)
                fT1 = sb.tile([128, 7, C], bf16, tag="fT1", bufs=2)
                fT2 = sb.tile([128, 7, C], bf16, tag="fT2", bufs=2)
                # chunked convert+transpose so each XBAR dma starts as soon
                # as its chunk is converted (f2 chunks first)
                for j in range(4):
                    nc.scalar.copy(f2w[:, j, :N], fB[:, j, :])
                    nc.sync.dma_start_transpose(
                        fT2[:, :, j * 128:(j + 1) * 128], f2w[:, j, :])
                f1q = sb.tile([128, 4, N], fp8, tag="f1q", bufs=2)
                for j in range(4):
                    nc.scalar.copy(f1w[:, j, :N], fA[:, j, :])
                    nc.sync.dma_start_transpose(
                        fT1[:, :, j * 128:(j + 1) * 128], f1w[:, j, :])
                    nc.scalar.copy(f1q[:, j, :], fA[:, j, :])

                # norms via one-pass bn_stats over c (per n-tile):
                # ssq = M2_even + 256*mean_even^2 + M2_odd + 256*mean_odd^2
                ssq = sb.tile([128, 16], f32, tag="ssq", bufs=2)
                nc.vector.memset(ssq[:], 1.0)  # keep pad lanes finite
                s = sb.tile([128, 16], f32, tag="s", bufs=2)
                r = sb.tile([128, 16], f32, tag="r", bufs=2)

                def norms(fT, col0):
                    st = sb.tile([128, 7, 6], f32, tag="st", bufs=2)
                    nc.vector.memset(st[:], 1.0)  # pad lanes of t=6
                    for t, (noff, nsz) in enumerate(NT):
                        nc.vector.bn_stats(st[:nsz, t, :], fT[:nsz, t, :])
                    w1 = sb.tile([128, 7], f32, tag="w1", bufs=2)
                    nc.vector.tensor_mul(w1[:], st[:, :, 1], st[:, :, 1])
                    w2 = sb.tile([128, 7], f32, tag="w2", bufs=2)
                    nc.vector.tensor_mul(w2[:], st[:, :, 4], st[:, :, 4])
                    w3 = sb.tile([128, 7], f32, tag="w3", bufs=2)
                    nc.vector.tensor_add(w3[:], w1[:], w2[:])
                    w4 = sb.tile([128, 7], f32, tag="w4", bufs=2)
                    nc.vector.tensor_add(w4[:], st[:, :, 2], st[:, :, 5])
                    nc.vector.scalar_tensor_tensor(
                        out=ssq[:, col0:col0 + 7], in0=w3[:], scalar=256.0,
                        in1=w4[:], op0=ALU.mult, op1=ALU.add)

                # f2 norm path -> r2 row -> broadcast -> prescale
                norms2 = norms  # alias for clarity
                norms2(fT2, 8)
                nc.scalar.sqrt(s[:, 8:15], ssq[:, 8:15])
                nc.vector.reciprocal(r[:, 8:15], s[:, 8:15])
                prow = ps.tile([1, N], f32, tag="big", bufs=4)
                for t, (moff, msz) in enumerate(NT):
                    nc.tensor.transpose(
                        prow[:1, moff:moff + msz], r[:msz, 8 + t:9 + t],
                        ident[:msz, :msz])
                r2row = sb.tile([1, N], bf16, tag="rowb", bufs=4)
                nc.vector.tensor_scalar_mul(r2row[:1, :], prow[:1, :], 32.0)
                r2B = sb.tile([128, N], bf16, tag="r2B", bufs=2)
                nc.gpsimd.partition_broadcast(r2B[:], r2row[:1, :])
                f2s = sb.tile([128, 4, N], fp8, tag="f2s", bufs=2)
                for j in range(4):
                    nc.vector.tensor_mul(f2s[:, j, :], f2w[:, j, :N], r2B[:])

                # f1 norm path -> negated per-partition exp scale
                norms(fT1, 0)
                nc.scalar.sqrt(s[:, 0:7], ssq[:, 0:7])
                nc.vector.reciprocal(r[:, 0:7], s[:, 0:7])
                r1n = sb.tile([128, 8], f32, tag="r1n", bufs=2)
                nc.vector.tensor_scalar_mul(r1n[:, :7], r[:, :7], -1.0 / 32.0)
                return f1q, fT1, fT2, r1n, f2s

            state = prep(0)
            for b in range(nbatch):
                f1q, fT1, fT2, r1n, f2s = state

                # ---- gram + exp (rowsum fused via accum) ----
                rsc = sb.tile([128, 8], f32, tag="rsc", bufs=2)
                nc.vector.memset(rsc[:], 1.0)  # pad lanes for col reciprocal
                E = sb.tile([128, 7, NPAD], bf16, tag="E", bufs=1)
                nc.gpsimd.memset(E[:, :, N:], 0.0)   # pad cols (XBAR reads)
                nc.gpsimd.memset(E[:, 6, :N], 0.0)   # rows 16+ of tile 6 stay 0
                for t, (noff, nsz) in enumerate(NT):
                    G = ps.tile([128, N], f32, tag="big", bufs=4)
                    for g in range(2):
                        for hoff, hsz in HALVES:
                            nc.tensor.matmul(
                                G[:nsz, hoff:hoff + hsz],
                                f1q[:, 2 * g:2 * g + 2, noff:noff + nsz],
                                f2s[:, 2 * g:2 * g + 2, hoff:hoff + hsz],
                                start=(g == 0), stop=(g == 1),
                                perf_mode=mybir.MatmulPerfMode.DoubleRow)
                    nc.scalar.activation(
                        out=E[:nsz, t, :N], in_=G[:nsz, :N], func=AF.Exp,
                        scale=r1n[:nsz, t:t + 1],
                        accum_out=rsc[:nsz, t:t + 1])

                # ---- ET[p, t, n] = E[n, 128t+p] via XBAR dma transpose ----
                ET = sb.tile([128, 7, NPAD], bf16, tag="ET", bufs=1)
                for u in range(7):
                    nc.sync.dma_start_transpose(
                        ET[:, :, u * 128:(u + 1) * 128], E[:, u, :])

                # ---- prefetch next batch's inputs/norms during our matmuls ----
                if b + 1 < nbatch:
                    state = prep(b + 1)

                # ---- rowsum/colsum scale rows. High priority: tiny work
                # that gates the O-multiplies; don't let next-batch prep
                # starve it on DVE/ACT.
                with tc.high_priority():
                    rr = sb.tile([128, 8], f32, tag="rr", bufs=2)
                    nc.vector.reciprocal(rr[:], rsc[:])
                    rsp = ps.tile([1, N], f32, tag="big", bufs=4)
                    for t, (noff, nsz) in enumerate(NT):
                        nc.tensor.transpose(
                            rsp[:1, noff:noff + nsz], rr[:nsz, t:t + 1],
                            ident[:nsz, :nsz])
                    rsB = row_to_bcast(rsp[:1, :N], "rs")

                    # colsum(E) row via ones-matmul -> bcast of 0.001/cs
                    csp = ps.tile([1, N], f32, tag="big", bufs=4)
                    for t, (noff, nsz) in enumerate(NT):
                        for hoff, hsz in HALVES:
                            nc.tensor.matmul(
                                csp[:1, hoff:hoff + hsz], onesb[:nsz, :1],
                                E[:nsz, t, hoff:hoff + hsz],
                                start=(t == 0), stop=(t == 6))
                    csr = sb.tile([1, N], f32, tag="csr", bufs=2)
                    nc.vector.reciprocal_approx_fast(out=csr[:1, :],
                                                     in_=csp[:1, :N])
                    rcB = row_to_bcast(csr[:1, :], "rc")

                # ---- new_fm2: P2[c,m] = sum_n f1[c,n] E[n,m]; O2 = P2*rcB ----
                for ci in range(4):
                    P = ps.tile([128, N], f32, tag="big", bufs=4)
                    for t, (noff, nsz) in enumerate(NT):
                        for hoff, hsz in HALVES:
                            nc.tensor.matmul(
                                P[:, hoff:hoff + hsz],
                                fT1[:nsz, t, ci * 128:(ci + 1) * 128],
                                E[:nsz, t, hoff:hoff + hsz],
                                start=(t == 0), stop=(t == 6))
                    with tc.high_priority():
                        O2 = sb.tile([128, N], f32, tag="O", bufs=4)
                        nc.vector.tensor_mul(O2[:], P[:], rcB[:])
                        nc.sync.dma_start(
                            out=o2[b, ci * 128:(ci + 1) * 128, :], in_=O2[:])

                # ---- new_fm1: P1[c,n] = sum_m f2[c,m] ET[m,n]; O1 = P1*rsB ----
                for ci in range(4):
                    P = ps.tile([128, N], f32, tag="big", bufs=4)
                    for t, (moff, msz) in enumerate(NT):
                        for hoff, hsz in HALVES:
                            nc.tensor.matmul(
                                P[:, hoff:hoff + hsz],
                                fT2[:msz, t, ci * 128:(ci + 1) * 128],
                                ET[:msz, t, hoff:hoff + hsz],
                                start=(t == 0), stop=(t == 6))
                    with tc.high_priority():
                        O1 = sb.tile([128, N], f32, tag="O", bufs=4)
                        nc.vector.tensor_mul(O1[:], P[:], rsB[:])
                        nc.sync.dma_start(
                            out=o1[b, ci * 128:(ci + 1) * 128, :], in_=O1[:])

    nc.compile()
    _BUILT[key] = nc
    return nc


def _run(fm1, fm2, trace=False):
    from concourse.bass_utils import run_bass_kernel_spmd

    fm1 = np.ascontiguousarray(np.asarray(fm1, np.float32).reshape(B_TOTAL, C, N))
    fm2 = np.ascontiguousarray(np.asarray(fm2, np.float32).reshape(B_TOTAL, C, N))
    nc = _build(B_PER_CORE)
    f1s = fm1.reshape(N_CORES, B_PER_CORE, C, N)
    f2s = fm2.reshape(N_CORES, B_PER_CORE, C, N)
    in_maps = [
        {"fm1": np.ascontiguousarray(f1s[i]), "fm2": np.ascontiguousarray(f2s[i])}
        for i in range(N_CORES)
    ]
    res = run_bass_kernel_spmd(nc, in_maps, core_ids=list(range(N_CORES)),
                               trace=trace)
    out1 = np.concatenate([res.results[i]["o1"] for i in range(N_CORES)], axis=0)
    out2 = np.concatenate([res.results[i]["o2"] for i in range(N_CORES)], axis=0)
    out1 = out1.reshape(B_TOTAL, C, 28, 28).astype(np.float32)
    out2 = out2.reshape(B_TOTAL, C, 28, 28).astype(np.float32)
    return (out1, out2), res


def kernel(fm1, fm2):
    (out1, out2), _ = _run(fm1, fm2)
    return out1, out2


# revision 22
# speedup vs baseline: 1.0584x; 1.0584x over previous
# Trainium2 Bass kernel for nn_FDM_3899830304921 (feature-map cosine-sim
# dual-softmax transport), data-parallel over the batch dim on 8 NeuronCores.
#
# Math per batch (c=512, n=m=784):
#   r1[n] = 1/||f1_:n||, r2[m] = 1/||f2_:m||
#   E[n,m]   = exp(-r1[n] * sum_c f1[c,n] * (f2[c,m]*r2[m]))   (= exp(-cos))
#   new_fm2  = (f1 @ E) * (0.001/colsum(E))      (softmax over n folded in)
#   new_fm1  = (f2 @ E^T) * (0.001/rowsum(E))    (softmax over m folded in)
#
# Engine assignment (per batch):
#   PE:   gram (f1^T f2), colsum ones-matmul, P1/P2 output matmuls,
#         14 tiny column->row transposes. All matmul operands bf16.
#   DMA:  IO + all big transposes (fT1, fT2, E^T) via the XBAR
#         dma_start_transpose path (2-byte dtype, 16x128 tiles).
#   ACT:  f32->bf16 input conversion, exp (w/ fused rowsum accumulation),
#         sqrt, tiny row copies. Only Exp+Sqrt use act tables.
#   DVE:  fused square+reduce for norms (tensor_tensor_reduce),
#         reciprocals, f2 prescale, output postscale multiplies.
#   Pool: pad memsets + partition_broadcast of the three scale rows.
#
# n/m are padded 784->896 (7x128) only where the XBAR transpose needs
# multiples of 128; pad lanes are zeroed or provably never read.
import sys

if "/opt/trn_rl_repo" not in sys.path:
    sys.path.insert(0, "/opt/trn_rl_repo")

import numpy as np

B_TOTAL = 32
B_PER_CORE = 4
N_CORES = 8
C = 512
N = 784  # 28*28, both spatial dims
NPAD = 896  # 7*128, for XBAR dma transposes
FACTOR = 0.001

# n (and m) tiling: 6 tiles of 128 + one of 16
NT = [(0, 128), (128, 128), (256, 128), (384, 128), (512, 128), (640, 128), (768, 16)]
# free-dim split of 784 into PSUM-bank-sized pieces
HALVES = [(0, 512), (512, 272)]

_BUILT = {}


def _build(nbatch, enable_asserts=False):
    key = (nbatch, enable_asserts)
    if key in _BUILT:
        return _BUILT[key]

    import concourse.bass as bass
    import concourse.tile as tile
    from concourse import bacc, mybir
    from concourse.masks import make_identity

    f32 = mybir.dt.float32
    bf16 = mybir.dt.bfloat16
    fp8 = mybir.dt.float8e4
    AF = mybir.ActivationFunctionType
    ALU = mybir.AluOpType

    nc = bacc.Bacc("TRN2", target_bir_lowering=False, debug=False,
                   enable_asserts=enable_asserts, num_devices=N_CORES)
    fm1 = nc.dram_tensor("fm1", [nbatch, C, N], f32, kind="ExternalInput").ap()
    fm2 = nc.dram_tensor("fm2", [nbatch, C, N], f32, kind="ExternalInput").ap()
    o1 = nc.dram_tensor("o1", [nbatch, C, N], f32, kind="ExternalOutput").ap()
    o2 = nc.dram_tensor("o2", [nbatch, C, N], f32, kind="ExternalOutput").ap()

    with tile.TileContext(nc) as tc:
        with (
            tc.tile_pool(name="sb", bufs=2) as sb,
            tc.tile_pool(name="ps", bufs=4, space="PSUM") as ps,
        ):
            ident = sb.tile([128, 128], f32, tag="ident", bufs=1)
            make_identity(nc, ident[:])
            onesb = sb.tile([128, 1], bf16, tag="onesb", bufs=1)
            nc.vector.memset(onesb[:], 1.0)

            def row_to_bcast(rcp_row_ps, tagstem):
                """[1,N] PSUM row holding 1/denom -> broadcast FACTOR/denom
                to [128,N] bf16 (FACTOR folded into the copy scale)."""
                rb = sb.tile([1, N], bf16, tag="rowb", bufs=4)
                nc.scalar.mul(rb[:1, :], rcp_row_ps, FACTOR)
                out = sb.tile([128, N], bf16, tag=tagstem + "B", bufs=2)
                nc.gpsimd.partition_broadcast(out[:], rb[:1, :])
                return out

            def prep(b):
                """Load batch b, convert to bf16, transpose via XBAR DMA,
                compute norms and the r2-prescaled f2. Emitted one batch
                ahead so the chain overlaps the previous batch's matmuls."""
                fA = sb.tile([128, 4, N], f32, tag="fA", bufs=2)
                nc.sync.dma_start(
                    out=fA[:], in_=fm1[b].rearrange("(t p) n -> p t n", p=128))
                fB = sb.tile([128, 4, N], f32, tag="fB", bufs=2)
                nc.sync.dma_start(
                    out=fB[:], in_=fm2[b].rearrange("(t p) n -> p t n", p=128))

                f1w = sb.tile([128, 4, NPAD], bf16, tag="f1w", bufs=2)
                f2w = sb.tile([128, 4, NPAD], bf16, tag="f2w", bufs=2)
                nc.gpsimd.memset(f1w[:, :, N:], 0.0)
                nc.gpsimd.memset(f2w[:, :, N:], 0.0)
                nc.scalar.copy(f1w[:, :, :N], fA[:])
                nc.scalar.copy(f2w[:, :, :N], fB[:])

                # fT[p, t, c] = f[c, 128t+p] via XBAR dma transpose
                fT1 = sb.tile([128, 7, C], bf16, tag="fT1", bufs=2)
                fT2 = sb.tile([128, 7, C], bf16, tag="fT2", bufs=2)
                for j in range(4):
                    nc.sync.dma_start_transpose(
                        fT1[:, :, j * 128:(j + 1) * 128], f1w[:, j, :])
                    nc.sync.dma_start_transpose(
                        fT2[:, :, j * 128:(j + 1) * 128], f2w[:, j, :])

                # norms via one-pass bn_stats over c (per n-tile):
                # ssq = M2_even + 256*mean_even^2 + M2_odd + 256*mean_odd^2
                ssq = sb.tile([128, 16], f32, tag="ssq", bufs=2)
                nc.vector.memset(ssq[:], 1.0)  # keep pad lanes finite
                for fT, col0 in ((fT1, 0), (fT2, 8)):
                    st = sb.tile([128, 7, 6], f32, tag="st", bufs=2)
                    nc.vector.memset(st[:], 1.0)  # pad lanes of t=6
                    for t, (noff, nsz) in enumerate(NT):
                        nc.vector.bn_stats(st[:nsz, t, :], fT[:nsz, t, :])
                    w1 = sb.tile([128, 7], f32, tag="w1", bufs=2)
                    nc.vector.tensor_mul(w1[:], st[:, :, 1], st[:, :, 1])
                    w2 = sb.tile([128, 7], f32, tag="w2", bufs=2)
                    nc.vector.tensor_mul(w2[:], st[:, :, 4], st[:, :, 4])
                    w3 = sb.tile([128, 7], f32, tag="w3", bufs=2)
                    nc.vector.tensor_add(w3[:], w1[:], w2[:])
                    w4 = sb.tile([128, 7], f32, tag="w4", bufs=2)
                    nc.vector.tensor_add(w4[:], st[:, :, 2], st[:, :, 5])
                    nc.vector.scalar_tensor_tensor(
                        out=ssq[:, col0:col0 + 7], in0=w3[:], scalar=256.0,
                        in1=w4[:], op0=ALU.mult, op1=ALU.add)
                s = sb.tile([128, 16], f32, tag="s", bufs=2)
                nc.scalar.sqrt(s[:], ssq[:])
                r = sb.tile([128, 16], f32, tag="r", bufs=2)
                nc.vector.reciprocal(r[:], s[:])
                r1n = sb.tile([128, 8], f32, tag="r1n", bufs=2)
                nc.vector.tensor_scalar_mul(r1n[:, :7], r[:, :7], -1.0)

                # r2 cols -> [1,N] row -> bf16 -> broadcast [128,N]
                prow = ps.tile([1, N], f32, tag="big", bufs=4)
                for t, (moff, msz) in enumerate(NT):
                    nc.tensor.transpose(
                        prow[:1, moff:moff + msz], r[:msz, 8 + t:9 + t],
                        ident[:msz, :msz])
                r2row = sb.tile([1, N], bf16, tag="rowb", bufs=4)
                nc.scalar.copy(r2row[:1, :], prow[:1, :])
                r2B = sb.tile([128, N], bf16, tag="r2B", bufs=2)
                nc.gpsimd.partition_broadcast(r2B[:], r2row[:1, :])

                # prescale: f2s[c,m] = f2[c,m] * r2[m]  (bf16)
                f2s = sb.tile([128, 4, N], bf16, tag="f2s", bufs=2)
                for j in range(4):
                    nc.vector.tensor_mul(f2s[:, j, :], f2w[:, j, :N], r2B[:])
                return f1w, fT1, fT2, r1n, f2s

            state = prep(0)
            for b in range(nbatch):
                f1w, fT1, fT2, r1n, f2s = state

                # ---- gram + exp (rowsum fused via accum) ----
                rsc = sb.tile([128, 8], f32, tag="rsc", bufs=2)
                nc.vector.memset(rsc[:], 1.0)  # pad lanes for col reciprocal
                E = sb.tile([128, 7, NPAD], bf16, tag="E", bufs=1)
                nc.gpsimd.memset(E[:, :, N:], 0.0)   # pad cols (XBAR reads)
                nc.gpsimd.memset(E[:, 6, :N], 0.0)   # rows 16+ of tile 6 stay 0
                for t, (noff, nsz) in enumerate(NT):
                    G = ps.tile([128, N], f32, tag="big", bufs=4)
                    for j in range(4):
                        for hoff, hsz in HALVES:
                            nc.tensor.matmul(
                                G[:nsz, hoff:hoff + hsz],
                                f1w[:, j, noff:noff + nsz],
                                f2s[:, j, hoff:hoff + hsz],
                                start=(j == 0), stop=(j == 3))
                    nc.scalar.activation(
                        out=E[:nsz, t, :N], in_=G[:nsz, :N], func=AF.Exp,
                        scale=r1n[:nsz, t:t + 1],
                        accum_out=rsc[:nsz, t:t + 1])

                # ---- ET[p, t, n] = E[n, 128t+p] via XBAR dma transpose ----
                ET = sb.tile([128, 7, NPAD], bf16, tag="ET", bufs=1)
                for u in range(7):
                    nc.sync.dma_start_transpose(
                        ET[:, :, u * 128:(u + 1) * 128], E[:, u, :])

                # ---- prefetch next batch's inputs/norms during our matmuls ----
                if b + 1 < nbatch:
                    state = prep(b + 1)

                # ---- rowsum/colsum scale rows. High priority: tiny work
                # that gates the O-multiplies; don't let next-batch prep
                # starve it on DVE/ACT.
                with tc.high_priority():
                    rr = sb.tile([128, 8], f32, tag="rr", bufs=2)
                    nc.vector.reciprocal(rr[:], rsc[:])
                    rsp = ps.tile([1, N], f32, tag="big", bufs=4)
                    for t, (noff, nsz) in enumerate(NT):
                        nc.tensor.transpose(
                            rsp[:1, noff:noff + nsz], rr[:nsz, t:t + 1],
                            ident[:nsz, :nsz])
                    rsB = row_to_bcast(rsp[:1, :N], "rs")

                    # colsum(E) row via ones-matmul -> bcast of 0.001/cs
                    csp = ps.tile([1, N], f32, tag="big", bufs=4)
                    for t, (noff, nsz) in enumerate(NT):
                        for hoff, hsz in HALVES:
                            nc.tensor.matmul(
                                csp[:1, hoff:hoff + hsz], onesb[:nsz, :1],
                                E[:nsz, t, hoff:hoff + hsz],
                                start=(t == 0), stop=(t == 6))
                    csr = sb.tile([1, N], f32, tag="csr", bufs=2)
                    nc.vector.reciprocal_approx_fast(out=csr[:1, :],
                                                     in_=csp[:1, :N])
                    rcB = row_to_bcast(csr[:1, :], "rc")

                # ---- new_fm2: P2[c,m] = sum_n f1[c,n] E[n,m]; O2 = P2*rcB ----
                for ci in range(4):
                    P = ps.tile([128, N], f32, tag="big", bufs=4)
                    for t, (noff, nsz) in enumerate(NT):
                        for hoff, hsz in HALVES:
                            nc.tensor.matmul(
                                P[:, hoff:hoff + hsz],
                                fT1[:nsz, t, ci * 128:(ci + 1) * 128],
                                E[:nsz, t, hoff:hoff + hsz],
                                start=(t == 0), stop=(t == 6))
                    with tc.high_priority():
                        O2 = sb.tile([128, N], f32, tag="O", bufs=4)
                        nc.vector.tensor_mul(O2[:], P[:], rcB[:])
                        nc.sync.dma_start(
                            out=o2[b, ci * 128:(ci + 1) * 128, :], in_=O2[:])

                # ---- new_fm1: P1[c,n] = sum_m f2[c,m] ET[m,n]; O1 = P1*rsB ----
                for ci in range(4):
                    P = ps.tile([128, N], f32, tag="big", bufs=4)
                    for t, (moff, msz) in enumerate(NT):
                        for hoff, hsz in HALVES:
                            nc.tensor.matmul(
                                P[:, hoff:hoff + hsz],
                                fT2[:msz, t, ci * 128:(ci + 1) * 128],
                                ET[:msz, t, hoff:hoff + hsz],
                                start=(t == 0), stop=(t == 6))
                    with tc.high_priority():
                        O1 = sb.tile([128, N], f32, tag="O", bufs=4)
                        nc.vector.tensor_mul(O1[:], P[:], rsB[:])
                        nc.sync.dma_start(
                            out=o1[b, ci * 128:(ci + 1) * 128, :], in_=O1[:])

    nc.compile()
    _BUILT[key] = nc
    return nc


def _run(fm1, fm2, trace=False):
    from concourse.bass_utils import run_bass_kernel_spmd

    fm1 = np.ascontiguousarray(np.asarray(fm1, np.float32).reshape(B_TOTAL, C, N))
    fm2 = np.ascontiguousarray(np.asarray(fm2, np.float32).reshape(B_TOTAL, C, N))
    nc = _build(B_PER_CORE)
    f1s = fm1.reshape(N_CORES, B_PER_CORE, C, N)
    f2s = fm2.reshape(N_CORES, B_PER_CORE, C, N)
    in_maps = [
        {"fm1": np.ascontiguousarray(f1s[i]), "fm2": np.ascontiguousarray(f2s[i])}
        for i in range(N_CORES)
    ]
    res = run_bass_kernel_spmd(nc, in_maps, core_ids=list(range(N_CORES)),
                               trace=trace)
    out1 = np.concatenate([res.results[i]["o1"] for i in range(N_CORES)], axis=0)
    out2 = np.concatenate([res.results[i]["o2"] for i in range(N_CORES)], axis=0)
    out1 = out1.reshape(B_TOTAL, C, 28, 28).astype(np.float32)
    out2 = out2.reshape(B_TOTAL, C, 28, 28).astype(np.float32)
    return (out1, out2), res


def kernel(fm1, fm2):
    (out1, out2), _ = _run(fm1, fm2)
    return out1, out2


# revision 25
# speedup vs baseline: 1.2762x; 1.2057x over previous
# Trainium2 Bass kernel for nn_FDM_3899830304921 (feature-map cosine-sim
# dual-softmax transport), data-parallel over the batch dim on 8 NeuronCores.
#
# Math per batch (c=512, n=m=784):
#   r1[n] = 1/||f1_:n||, r2[m] = 1/||f2_:m||
#   E[n,m]   = exp(-r1[n] * sum_c f1[c,n] * (f2[c,m]*r2[m]))   (= exp(-cos))
#   new_fm2  = (f1 @ E) * (0.001/colsum(E))      (softmax over n folded in)
#   new_fm1  = (f2 @ E^T) * (0.001/rowsum(E))    (softmax over m folded in)
#
# Engine assignment (per batch):
#   PE:   gram (f1^T f2), colsum ones-matmul, P1/P2 output matmuls,
#         14 tiny column->row transposes. All matmul operands bf16.
#   DMA:  IO + all big transposes (fT1, fT2, E^T) via the XBAR
#         dma_start_transpose path (2-byte dtype, 16x128 tiles).
#   ACT:  f32->bf16 input conversion, exp (w/ fused rowsum accumulation),
#         sqrt, tiny row copies. Only Exp+Sqrt use act tables.
#   DVE:  fused square+reduce for norms (tensor_tensor_reduce),
#         reciprocals, f2 prescale, output postscale multiplies.
#   Pool: pad memsets + partition_broadcast of the three scale rows.
#
# n/m are padded 784->896 (7x128) only where the XBAR transpose needs
# multiples of 128; pad lanes are zeroed or provably never read.
import sys

if "/opt/trn_rl_repo" not in sys.path:
    sys.path.insert(0, "/opt/trn_rl_repo")

import numpy as np

B_TOTAL = 32
B_PER_CORE = 4
N_CORES = 8
C = 512
N = 784  # 28*28, both spatial dims
NPAD = 896  # 7*128, for XBAR dma transposes
FACTOR = 0.001

# n (and m) tiling: 6 tiles of 128 + one of 16
NT = [(0, 128), (128, 128), (256, 128), (384, 128), (512, 128), (640, 128), (768, 16)]
# free-dim split of 784 into PSUM-bank-sized pieces
HALVES = [(0, 512), (512, 272)]

_BUILT = {}


def _build(nbatch, enable_asserts=False):
    key = (nbatch, enable_asserts)
    if key in _BUILT:
        return _BUILT[key]

    import concourse.bass as bass
    import concourse.tile as tile
    from concourse import bacc, mybir
    from concourse.masks import make_identity

    f32 = mybir.dt.float32
    bf16 = mybir.dt.bfloat16
    fp8 = mybir.dt.float8e4
    AF = mybir.ActivationFunctionType
    ALU = mybir.AluOpType

    nc = bacc.Bacc("TRN2", target_bir_lowering=False, debug=False,
                   enable_asserts=enable_asserts, num_devices=N_CORES)
    fm1 = nc.dram_tensor("fm1", [nbatch, C, N], f32, kind="ExternalInput").ap()
    fm2 = nc.dram_tensor("fm2", [nbatch, C, N], f32, kind="ExternalInput").ap()
    o1 = nc.dram_tensor("o1", [nbatch, C, N], f32, kind="ExternalOutput").ap()
    o2 = nc.dram_tensor("o2", [nbatch, C, N], f32, kind="ExternalOutput").ap()

    with tile.TileContext(nc) as tc:
        with (
            tc.tile_pool(name="sb", bufs=2) as sb,
            tc.tile_pool(name="ps", bufs=4, space="PSUM") as ps,
        ):
            ident = sb.tile([128, 128], f32, tag="ident", bufs=1)
            make_identity(nc, ident[:])
            onesb = sb.tile([128, 1], bf16, tag="onesb", bufs=1)
            nc.vector.memset(onesb[:], 1.0)

            def row_to_bcast(rcp_row_ps, tagstem):
                """[1,N] PSUM row holding 1/denom -> broadcast FACTOR/denom
                to [128,N] bf16 (FACTOR folded into the copy scale)."""
                rb = sb.tile([1, N], bf16, tag="rowb", bufs=4)
                nc.scalar.mul(rb[:1, :], rcp_row_ps, FACTOR)
                out = sb.tile([128, N], bf16, tag=tagstem + "B", bufs=2)
                nc.gpsimd.partition_broadcast(out[:], rb[:1, :])
                return out

            def prep(b):
                """Load batch b, convert to bf16, transpose via XBAR DMA,
                compute norms and the r2-prescaled f2. Emitted one batch
                ahead so the chain overlaps the previous batch's matmuls."""
                fA = sb.tile([128, 4, N], f32, tag="fA", bufs=2)
                nc.sync.dma_start(
                    out=fA[:], in_=fm1[b].rearrange("(t p) n -> p t n", p=128))
                fB = sb.tile([128, 4, N], f32, tag="fB", bufs=2)
                nc.sync.dma_start(
                    out=fB[:], in_=fm2[b].rearrange("(t p) n -> p t n", p=128))

                f1w = sb.tile([128, 4, NPAD], bf16, tag="f1w", bufs=2)
                f2w = sb.tile([128, 4, NPAD], bf16, tag="f2w", bufs=2)
                nc.gpsimd.memset(f1w[:, :, N:], 0.0)
                nc.gpsimd.memset(f2w[:, :, N:], 0.0)
                nc.scalar.copy(f1w[:, :, :N], fA[:])
                nc.scalar.copy(f2w[:, :, :N], fB[:])

                # fT[p, t, c] = f[c, 128t+p] via XBAR dma transpose
                fT1 = sb.tile([128, 7, C], bf16, tag="fT1", bufs=2)
                fT2 = sb.tile([128, 7, C], bf16, tag="fT2", bufs=2)
                for j in range(4):
                    nc.sync.dma_start_transpose(
                        fT1[:, :, j * 128:(j + 1) * 128], f1w[:, j, :])
                    nc.sync.dma_start_transpose(
                        fT2[:, :, j * 128:(j + 1) * 128], f2w[:, j, :])

                # ---- norms via ones-matmul over f^2: no dependency on
                # the fT transposes, so the scale rows are ready ~7us after
                # the loads and the next gram is never prep-gated.
                # f2: r2 as a row (for the prescale broadcast).
                fsq2 = sb.tile([128, 4, N], bf16, tag="fsq", bufs=1)
                nc.vector.tensor_mul(fsq2[:], fB[:], fB[:])
                sq2p = ps.tile([1, N], f32, tag="big", bufs=4)
                for j in range(4):
                    for hoff, hsz in HALVES:
                        nc.tensor.matmul(
                            sq2p[:1, hoff:hoff + hsz], onesb[:, :1],
                            fsq2[:, j, hoff:hoff + hsz],
                            start=(j == 0), stop=(j == 3))
                s2row = sb.tile([1, N], f32, tag="srow", bufs=4)
                nc.scalar.sqrt(s2row[:1, :], sq2p[:1, :])
                r2rowf = sb.tile([1, N], f32, tag="rrow", bufs=4)
                nc.vector.reciprocal_approx_fast(out=r2rowf[:1, :],
                                                 in_=s2row[:1, :])
                r2row = sb.tile([1, N], bf16, tag="rowb", bufs=4)
                nc.vector.tensor_copy(r2row[:1, :], r2rowf[:1, :])
                r2B = sb.tile([128, N], bf16, tag="r2B", bufs=2)
                nc.gpsimd.partition_broadcast(r2B[:], r2row[:1, :])
                f2s = sb.tile([128, 4, N], bf16, tag="f2s", bufs=2)
                for j in range(4):
                    nc.vector.tensor_mul(f2s[:, j, :], f2w[:, j, :N], r2B[:])

                # f1: -r1 as per-partition columns (for the exp scale).
                fsq1 = sb.tile([128, 4, N], bf16, tag="fsq", bufs=1)
                nc.vector.tensor_mul(fsq1[:], fA[:], fA[:])
                sq1p = ps.tile([1, N], f32, tag="big", bufs=4)
                for j in range(4):
                    for hoff, hsz in HALVES:
                        nc.tensor.matmul(
                            sq1p[:1, hoff:hoff + hsz], onesb[:, :1],
                            fsq1[:, j, hoff:hoff + hsz],
                            start=(j == 0), stop=(j == 3))
                s1row = sb.tile([1, N], f32, tag="srow", bufs=4)
                nc.scalar.sqrt(s1row[:1, :], sq1p[:1, :])
                r1rowf = sb.tile([1, N], f32, tag="rrow", bufs=4)
                nc.vector.reciprocal_approx_fast(out=r1rowf[:1, :],
                                                 in_=s1row[:1, :])
                r1p = ps.tile([128, 8], f32, tag="big", bufs=4)
                nc.vector.memset(r1p[:], 1.0)
                for t, (noff, nsz) in enumerate(NT):
                    nc.tensor.transpose(
                        r1p[:nsz, t:t + 1], r1rowf[:1, noff:noff + nsz],
                        ident[:1, :1])
                r1n = sb.tile([128, 8], f32, tag="r1n", bufs=2)
                nc.scalar.mul(r1n[:], r1p[:], -1.0)
                return f1w, fT1, fT2, r1n, f2s

            state = prep(0)
            for b in range(nbatch):
                f1w, fT1, fT2, r1n, f2s = state

                # ---- gram + exp (rowsum fused via accum) ----
                rsc = sb.tile([128, 8], f32, tag="rsc", bufs=2)
                nc.vector.memset(rsc[:], 1.0)  # pad lanes for col reciprocal
                E = sb.tile([128, 7, NPAD], bf16, tag="E", bufs=1)
                nc.gpsimd.memset(E[:, :, N:], 0.0)   # pad cols (XBAR reads)
                nc.gpsimd.memset(E[:, 6, :N], 0.0)   # rows 16+ of tile 6 stay 0
                for t, (noff, nsz) in enumerate(NT):
                    G = ps.tile([128, N], f32, tag="big", bufs=4)
                    for j in range(4):
                        for hoff, hsz in HALVES:
                            nc.tensor.matmul(
                                G[:nsz, hoff:hoff + hsz],
                                f1w[:, j, noff:noff + nsz],
                                f2s[:, j, hoff:hoff + hsz],
                                start=(j == 0), stop=(j == 3))
                    nc.scalar.activation(
                        out=E[:nsz, t, :N], in_=G[:nsz, :N], func=AF.Exp,
                        scale=r1n[:nsz, t:t + 1],
                        accum_out=rsc[:nsz, t:t + 1])

                # ---- ET[p, t, n] = E[n, 128t+p] via XBAR dma transpose ----
                ET = sb.tile([128, 7, NPAD], bf16, tag="ET", bufs=1)
                for u in range(7):
                    nc.sync.dma_start_transpose(
                        ET[:, :, u * 128:(u + 1) * 128], E[:, u, :])

                # ---- prefetch next batch's inputs/norms during our matmuls ----
                if b + 1 < nbatch:
                    state = prep(b + 1)

                # ---- rowsum/colsum scale rows. High priority: tiny work
                # that gates the O-multiplies; don't let next-batch prep
                # starve it on DVE/ACT.
                with tc.high_priority():
                    rr = sb.tile([128, 8], f32, tag="rr", bufs=2)
                    nc.vector.reciprocal(rr[:], rsc[:])
                    rsp = ps.tile([1, N], f32, tag="big", bufs=4)
                    for t, (noff, nsz) in enumerate(NT):
                        nc.tensor.transpose(
                            rsp[:1, noff:noff + nsz], rr[:nsz, t:t + 1],
                            ident[:nsz, :nsz])
                    rsB = row_to_bcast(rsp[:1, :N], "rs")

                    # colsum(E) row via ones-matmul -> bcast of 0.001/cs
                    csp = ps.tile([1, N], f32, tag="big", bufs=4)
                    for t, (noff, nsz) in enumerate(NT):
                        for hoff, hsz in HALVES:
                            nc.tensor.matmul(
                                csp[:1, hoff:hoff + hsz], onesb[:nsz, :1],
                                E[:nsz, t, hoff:hoff + hsz],
                                start=(t == 0), stop=(t == 6))
                    csr = sb.tile([1, N], f32, tag="csr", bufs=2)
                    nc.vector.reciprocal_approx_fast(out=csr[:1, :],
                                                     in_=csp[:1, :N])
                    rcB = row_to_bcast(csr[:1, :], "rc")

                # ---- new_fm2: P2[c,m] = sum_n f1[c,n] E[n,m]; O2 = P2*rcB ----
                for ci in range(4):
                    P = ps.tile([128, N], f32, tag="big", bufs=4)
                    for t, (noff, nsz) in enumerate(NT):
                        for hoff, hsz in HALVES:
                            nc.tensor.matmul(
                                P[:, hoff:hoff + hsz],
                                fT1[:nsz, t, ci * 128:(ci + 1) * 128],
                                E[:nsz, t, hoff:hoff + hsz],
                                start=(t == 0), stop=(t == 6))
                    with tc.high_priority():
                        O2 = sb.tile([128, N], f32, tag="O", bufs=4)
                        nc.vector.tensor_mul(O2[:], P[:], rcB[:])
                        nc.sync.dma_start(
                            out=o2[b, ci * 128:(ci + 1) * 128, :], in_=O2[:])

                # ---- new_fm1: P1[c,n] = sum_m f2[c,m] ET[m,n]; O1 = P1*rsB ----
                for ci in range(4):
                    P = ps.tile([128, N], f32, tag="big", bufs=4)
                    for t, (moff, msz) in enumerate(NT):
                        for hoff, hsz in HALVES:
                            nc.tensor.matmul(
                                P[:, hoff:hoff + hsz],
                                fT2[:msz, t, ci * 128:(ci + 1) * 128],
                                ET[:msz, t, hoff:hoff + hsz],
                                start=(t == 0), stop=(t == 6))
                    with tc.high_priority():
                        O1 = sb.tile([128, N], f32, tag="O", bufs=4)
                        nc.vector.tensor_mul(O1[:], P[:], rsB[:])
                        nc.sync.dma_start(
                            out=o1[b, ci * 128:(ci + 1) * 128, :], in_=O1[:])

    nc.compile()
    _BUILT[key] = nc
    return nc


def _run(fm1, fm2, trace=False):
    from concourse.bass_utils import run_bass_kernel_spmd

    fm1 = np.ascontiguousarray(np.asarray(fm1, np.float32).reshape(B_TOTAL, C, N))
    fm2 = np.ascontiguousarray(np.asarray(fm2, np.float32).reshape(B_TOTAL, C, N))
    nc = _build(B_PER_CORE)
    f1s = fm1.reshape(N_CORES, B_PER_CORE, C, N)
    f2s = fm2.reshape(N_CORES, B_PER_CORE, C, N)
    in_maps = [
        {"fm1": np.ascontiguousarray(f1s[i]), "fm2": np.ascontiguousarray(f2s[i])}
        for i in range(N_CORES)
    ]
    res = run_bass_kernel_spmd(nc, in_maps, core_ids=list(range(N_CORES)),
                               trace=trace)
    out1 = np.concatenate([res.results[i]["o1"] for i in range(N_CORES)], axis=0)
    out2 = np.concatenate([res.results[i]["o2"] for i in range(N_CORES)], axis=0)
    out1 = out1.reshape(B_TOTAL, C, 28, 28).astype(np.float32)
    out2 = out2.reshape(B_TOTAL, C, 28, 28).astype(np.float32)
    return (out1, out2), res


def kernel(fm1, fm2):
    (out1, out2), _ = _run(fm1, fm2)
    return out1, out2
